# revision 4
# baseline (speedup 1.0000x reference)
"""Masked dot-product attention (ESIM masked_softmax) Trainium2 Bass kernel.

Math (per batch):
    s   = q @ k^T                      [Lq, Lk]
    t   = s * m  (mask over k)         == q @ (k*m)^T
    p   = exp(t) * m / sum(exp(t) * m) (max-subtraction cancels; range is safe:
                                        |s| <= ~50 so exp fits fp32 comfortably)
    out = p @ v = (exp(t) @ (v*m)) / (exp(t) @ m)

Device layout (per core, 2 batches):
    - s is computed TRANSPOSED (k on partitions, q on free dim) so that
      exp(s^T) is directly the lhsT of the PV matmul: no O(Lq*Lk) transposes.
    - k*m and q are transposed once per batch via PE transposes ([128,128]
      tiles), with q duplicated into both partition halves and k-blocks
      packed in pairs so the K=64 score matmuls can be row-tiled two-at-a-time
      (full 128-row PE utilization).
    - The PV matmul uses [v*m | m] as the stationary operand: column 64 of the
      accumulated output is the softmax denominator for free.
    - out^T [65, Lq] is transposed back in 128-column chunks via PE transposes
      and normalized with a per-partition reciprocal multiply.

Precision modes for the two big matmuls (hardware truncates float32r operands
to fp22):
    S_MODE  = "f32r"  : 1 pass,  scores see fp22 operands  (~3.5e-4 max rel)
            = "3pass" : hi/lo split q/km, 3 f32r passes == full fp32 scores
    PV_MODE = "f32r" | "fp32"
"""

import os
import sys

import numpy as np

sys.path.insert(0, "/opt/trn_rl_repo")

import concourse.bacc as bacc
import concourse.bass as bass
import concourse.mybir as mybir
import concourse.tile as tile
from concourse import bass_utils
from concourse.masks import make_identity

B, LQ, LK, D = 16, 2048, 2048, 64
NCORES = 8
PB = B // NCORES  # batches per core
P = 128
NKB = LK // P  # 16 k-blocks
NQB = LQ // P  # 16 q-blocks

S_MODE = os.environ.get("ATT_S_MODE", "3pass")  # "f32r" | "3pass"
PV_MODE = os.environ.get("ATT_PV_MODE", "f32r")  # "f32r" | "fp32"

F32 = mybir.dt.float32
F32R = mybir.dt.float32r
EXP = mybir.ActivationFunctionType.Exp


def _attention_core(tc, q_d, k_d, v_d, m_d, o_d):
    """Emit the per-core program. All dram handles are per-core shards."""
    nc = tc.nc
    ctx_pools = []

    def pool(name, bufs):
        p = tc.alloc_tile_pool(name=name, bufs=bufs)
        ctx_pools.append(p)
        return p

    def psum_pool(name, bufs):
        p = tc.alloc_tile_pool(name=name, bufs=bufs, space="PSUM")
        ctx_pools.append(p)
        return p

    singles = pool("singles", 1)
    stage = pool("stage", 2)
    main = pool("main", 2)
    wtp = pool("wt", 8)
    outp = pool("outp", 2)
    smalls = pool("smalls", 4)

    ps_s = psum_pool("ps_s", 3)  # 3 x [128,1024] = 6 banks
    ps_pv = psum_pool("ps_pv", 1)  # 1 x [128,1024] = 2 banks

    ident = singles.tile([P, P], F32, tag="ident")
    make_identity(nc, ident)

    three = S_MODE == "3pass"

    for b in range(PB):
        # ---------------- load + mask-fold + transpose prep ----------------
        m_sb = stage.tile([P, NKB], F32, tag="m")
        nc.sync.dma_start(out=m_sb, in_=m_d[b].rearrange("(t p) -> p t", p=P))

        knat = stage.tile([P, NKB, D], F32, tag="knat")
        nc.sync.dma_start(out=knat, in_=k_d[b].rearrange("(t p) d -> p t d", p=P))
        vnat = stage.tile([P, NKB, D], F32, tag="vnat")
        nc.sync.dma_start(out=vnat, in_=v_d[b].rearrange("(t p) d -> p t d", p=P))
        qdup = stage.tile([P, NQB, 2, D], F32, tag="qdup")
        qsrc = q_d[b].rearrange("(t p) d -> p t d", p=P)
        nc.sync.dma_start(out=qdup[:, :, 0, :], in_=qsrc)
        nc.sync.dma_start(out=qdup[:, :, 1, :], in_=qsrc)

        km = stage.tile([P, NKB, D], F32, tag="km")
        vme = stage.tile([P, NKB, D + 1], F32R if PV_MODE == "f32r" else F32, tag="vme")
        for t in range(NKB):
            nc.vector.tensor_scalar_mul(km[:, t, :], knat[:, t, :], m_sb[:, t : t + 1])
            nc.vector.tensor_scalar_mul(
                vme[:, t, 0:D], vnat[:, t, :], m_sb[:, t : t + 1]
            )
        nc.vector.tensor_copy(vme[:, :, D], m_sb[:, :])

        # transposes: qT [128, 2048] (q dup over partition halves),
        # kmT [128, 8, 128] (k-block pairs packed over partition halves).
        # In 1-pass mode the PSUM->SBUF copy rounds to f32r directly; in
        # 3-pass mode keep fp32 and split hi/lo below.
        sdt = F32 if three else F32R
        qT = main.tile([P, LQ], sdt, tag="qT")
        for g in range(2):
            tr = ps_s.tile([P, 8 * P], F32, tag="s")
            for i in range(8):
                t = g * 8 + i
                nc.tensor.transpose(tr[:, i * P : (i + 1) * P], qdup[:, t], ident)
            nc.vector.tensor_copy(qT[:, g * 8 * P : (g + 1) * 8 * P], tr)

        kmT = main.tile([P, NKB // 2, P], sdt, tag="kmT")
        trk = ps_s.tile([P, 8 * P], F32, tag="s")
        for j in range(NKB // 2):
            nc.tensor.transpose(trk[:, j * P : (j + 1) * P], km[:, 2 * j : 2 * j + 2, :], ident)
        nc.vector.tensor_copy(kmT.rearrange("p a b -> p (a b)"), trk)

        if three:
            # hi = round_to_f32r(x); lo = round_to_f32r(x - hi). hi+lo == x to
            # ~fp45, so 3 f32r passes reproduce fp32 matmul precision.
            qTh = main.tile([P, LQ], F32R, tag="qTh")
            qTl = main.tile([P, LQ], F32R, tag="qTl")
            kmTh = main.tile([P, NKB // 2, P], F32R, tag="kmTh")
            kmTl = main.tile([P, NKB // 2, P], F32R, tag="kmTl")
            nc.vector.tensor_copy(qTh, qT)
            nc.vector.tensor_sub(qTl, qT, qTh.bitcast(F32))
            kmT_f = kmT.rearrange("p a b -> p (a b)")
            kmTh_f = kmTh.rearrange("p a b -> p (a b)")
            nc.vector.tensor_copy(kmTh_f, kmT_f)
            nc.vector.tensor_sub(
                kmTl.rearrange("p a b -> p (a b)"), kmT_f, kmTh_f.bitcast(F32)
            )

        out_sb = outp.tile([P, NQB, D], F32, tag="osb")

        # ---------------- main stream: S^T matmul -> exp -> PV ----------------
        for h in range(2):  # q halves of 1024
            pv = ps_pv.tile([P, 1024], F32, tag="pv")
            for j in range(NKB // 2):
                sA = ps_s.tile([P, 1024], F32, tag="s")
                sB = ps_s.tile([P, 1024], F32, tag="s")
                for c in range(2):  # 512-wide q slices
                    qs = slice(h * 1024 + c * 512, h * 1024 + (c + 1) * 512)
                    cs = slice(c * 512, (c + 1) * 512)
                    if three:
                        passes = [
                            (kmTh, qTh, True, False),
                            (kmTl, qTh, False, False),
                            (kmTh, qTl, False, True),
                        ]
                    else:
                        passes = [(kmT, qT, True, True)]
                    for kt, qt, st, sp in passes:
                        nc.tensor.matmul(
                            sA[:, cs], kt[0:64, j, :], qt[0:64, qs],
                            start=st, stop=sp, tile_position=(0, 0),
                        )
                        nc.tensor.matmul(
                            sB[:, cs], kt[64:128, j, :], qt[64:128, qs],
                            start=st, stop=sp, tile_position=(64, 0),
                        )
                wdt = F32R if PV_MODE == "f32r" else F32
                wA = wtp.tile([P, 1024], wdt, tag="wt")
                wB = wtp.tile([P, 1024], wdt, tag="wt")
                nc.scalar.activation(out=wA, in_=sA, func=EXP)
                nc.scalar.activation(out=wB, in_=sB, func=EXP)
                for c in range(2):
                    cs = slice(c * 512, (c + 1) * 512)
                    for kb, w in ((2 * j, wA), (2 * j + 1, wB)):
                        nc.tensor.matmul(
                            pv[0 : D + 1, cs], vme[:, kb, :], w[:, cs],
                            start=(kb == 0), stop=(kb == NKB - 1),
                        )

            # ---------------- drain: transpose back + normalize ----------------
            outT = outp.tile([D + 1, 1024], F32, tag="outT")
            nc.vector.tensor_copy(outT, pv[0 : D + 1, :])
            for qb in range(8):
                nat = ps_pv.tile([P, D + 1], F32, tag="pv")
                nc.tensor.transpose(
                    nat, outT[:, qb * P : (qb + 1) * P], ident[0 : D + 1, 0 : D + 1]
                )
                rc = smalls.tile([P, 1], F32, tag="rc")
                nc.vector.reciprocal(rc, nat[:, D : D + 1])
                nc.vector.tensor_scalar_mul(out_sb[:, h * 8 + qb, :], nat[:, 0:D], rc)

        nc.sync.dma_start(
            out=o_d[b].rearrange("(t p) d -> p t d", p=P), in_=out_sb
        )

    for p in reversed(ctx_pools):
        p.release()


_NC_CACHE = None


def _build_nc():
    global _NC_CACHE
    if _NC_CACHE is not None:
        return _NC_CACHE
    nc = bacc.Bacc(None, target_bir_lowering=False, debug=False)
    q_d = nc.dram_tensor("q", [PB, LQ, D], F32, kind="ExternalInput")
    k_d = nc.dram_tensor("k", [PB, LK, D], F32, kind="ExternalInput")
    v_d = nc.dram_tensor("v", [PB, LK, D], F32, kind="ExternalInput")
    m_d = nc.dram_tensor("m", [PB, LK], F32, kind="ExternalInput")
    o_d = nc.dram_tensor("out", [PB, LQ, D], F32, kind="ExternalOutput")
    with tile.TileContext(nc) as tc:
        _attention_core(tc, q_d, k_d, v_d, m_d, o_d)
    nc.compile()
    _NC_CACHE = nc
    return nc


def kernel(q, k, v, v_mask, _trace=False, _tmpdir=None):
    q = np.ascontiguousarray(q, dtype=np.float32)
    k = np.ascontiguousarray(k, dtype=np.float32)
    v = np.ascontiguousarray(v, dtype=np.float32)
    v_mask = np.ascontiguousarray(v_mask, dtype=np.float32)
    assert q.shape == (B, LQ, D), q.shape

    nc = _build_nc()
    in_maps = [
        {
            "q": q[i * PB : (i + 1) * PB],
            "k": k[i * PB : (i + 1) * PB],
            "v": v[i * PB : (i + 1) * PB],
            "m": v_mask[i * PB : (i + 1) * PB],
        }
        for i in range(NCORES)
    ]
    res = bass_utils.run_bass_kernel_spmd(
        nc, in_maps, core_ids=list(range(NCORES)), trace=_trace, tmpdir=_tmpdir
    )
    out = np.concatenate([r["out"] for r in res.results], axis=0)
    if _trace:
        kernel.last_results = res
    return out


# revision 7
# speedup vs baseline: 1.0815x; 1.0815x over previous
"""Masked dot-product attention (ESIM masked_softmax) Trainium2 Bass kernel.

Math (per batch):
    s   = q @ k^T                      [Lq, Lk]
    t   = s * m  (mask over k)         == q @ (k*m)^T
    p   = exp(t) * m / sum(exp(t) * m) (max-subtraction cancels; range is safe:
                                        |s| <= ~50 so exp fits fp32 comfortably)
    out = p @ v = (exp(t) @ (v*m)) / (exp(t) @ m)

Device layout (per core, 2 batches):
    - s is computed TRANSPOSED (k on partitions, q on free dim) so that
      exp(s^T) is directly the lhsT of the PV matmul: no O(Lq*Lk) transposes.
    - k*m and q are transposed once per batch via PE transposes ([128,128]
      tiles), with q duplicated into both partition halves and k-blocks
      packed in pairs so the K=64 score matmuls can be row-tiled two-at-a-time
      (full 128-row PE utilization).
    - The PV matmul uses [v*m | m] as the stationary operand: column 64 of the
      accumulated output is the softmax denominator for free.
    - out^T [65, Lq] is transposed back in 128-column chunks via PE transposes
      and normalized with a per-partition reciprocal multiply.

Precision modes for the two big matmuls (hardware truncates float32r operands
to fp22):
    S_MODE  = "f32r"  : 1 pass,  scores see fp22 operands  (~3.5e-4 max rel)
            = "3pass" : hi/lo split q/km, 3 f32r passes == full fp32 scores
    PV_MODE = "f32r" | "fp32"
"""

import os
import sys

import numpy as np

sys.path.insert(0, "/opt/trn_rl_repo")

import concourse.bacc as bacc
import concourse.bass as bass
import concourse.mybir as mybir
import concourse.tile as tile
from concourse import bass_utils
from concourse.masks import make_identity

B, LQ, LK, D = 16, 2048, 2048, 64
NCORES = 8
PB = B // NCORES  # batches per core
P = 128
NKB = LK // P  # 16 k-blocks
NQB = LQ // P  # 16 q-blocks

S_MODE = os.environ.get("ATT_S_MODE", "bf16_3p")  # "bf16_3p" | "f32r"
PV_MODE = os.environ.get("ATT_PV_MODE", "f32r")  # "f32r" | "fp32"

F32 = mybir.dt.float32
F32R = mybir.dt.float32r
BF16 = mybir.dt.bfloat16
EXP = mybir.ActivationFunctionType.Exp


def _attention_core(tc, q_d, k_d, v_d, m_d, o_d):
    """Emit the per-core program. All dram handles are per-core shards."""
    nc = tc.nc
    ctx_pools = []

    def pool(name, bufs):
        p = tc.alloc_tile_pool(name=name, bufs=bufs)
        ctx_pools.append(p)
        return p

    def psum_pool(name, bufs):
        p = tc.alloc_tile_pool(name=name, bufs=bufs, space="PSUM")
        ctx_pools.append(p)
        return p

    singles = pool("singles", 1)
    stage = pool("stage", 2)
    main = pool("main", 2)
    wtp = pool("wt", 8)
    outp = pool("outp", 2)
    smalls = pool("smalls", 4)

    ps_s = psum_pool("ps_s", 3)  # 3 x [128,1024] = 6 banks
    ps_pv = psum_pool("ps_pv", 1)  # 1 x [128,1024] = 2 banks

    ident = singles.tile([P, P], F32, tag="ident")
    make_identity(nc, ident)

    three = S_MODE == "bf16_3p"

    for b in range(PB):
        # ---------------- load + mask-fold + transpose prep ----------------
        m_sb = stage.tile([P, NKB], F32, tag="m")
        nc.sync.dma_start(out=m_sb, in_=m_d[b].rearrange("(t p) -> p t", p=P))

        knat = stage.tile([P, NKB, D], F32, tag="knat")
        nc.sync.dma_start(out=knat, in_=k_d[b].rearrange("(t p) d -> p t d", p=P))
        vnat = stage.tile([P, NKB, D], F32, tag="vnat")
        nc.sync.dma_start(out=vnat, in_=v_d[b].rearrange("(t p) d -> p t d", p=P))
        qdup = stage.tile([P, NQB, 2, D], F32, tag="qdup")
        qsrc = q_d[b].rearrange("(t p) d -> p t d", p=P)
        nc.sync.dma_start(out=qdup[:, :, 0, :], in_=qsrc)
        nc.sync.dma_start(out=qdup[:, :, 1, :], in_=qsrc)

        km = stage.tile([P, NKB, D], F32, tag="km")
        vme = stage.tile([P, NKB, D + 1], F32R if PV_MODE == "f32r" else F32, tag="vme")
        for t in range(NKB):
            nc.vector.tensor_scalar_mul(km[:, t, :], knat[:, t, :], m_sb[:, t : t + 1])
            nc.vector.tensor_scalar_mul(
                vme[:, t, 0:D], vnat[:, t, :], m_sb[:, t : t + 1]
            )
        nc.vector.tensor_copy(vme[:, :, D], m_sb[:, :])

        # transposes: qT [128, 2048] (q dup over partition halves),
        # kmT [128, 8, 128] (k-block pairs packed over partition halves).
        # In 1-pass mode the PSUM->SBUF copy rounds to f32r directly; in
        # 3-pass mode keep fp32 and split hi/lo below.
        sdt = F32 if three else F32R
        qT = main.tile([P, LQ], sdt, tag="qT")
        for g in range(2):
            tr = ps_s.tile([P, 8 * P], F32, tag="s")
            for i in range(8):
                t = g * 8 + i
                nc.tensor.transpose(tr[:, i * P : (i + 1) * P], qdup[:, t], ident)
            nc.vector.tensor_copy(qT[:, g * 8 * P : (g + 1) * 8 * P], tr)

        kmT = main.tile([P, NKB // 2, P], sdt, tag="kmT")
        trk = ps_s.tile([P, 8 * P], F32, tag="s")
        for j in range(NKB // 2):
            nc.tensor.transpose(trk[:, j * P : (j + 1) * P], km[:, 2 * j : 2 * j + 2, :], ident)
        nc.vector.tensor_copy(kmT.rearrange("p a b -> p (a b)"), trk)

        if three:
            # bf16 hi/lo split: hi = bf16(x), lo = bf16(x - hi). Three bf16
            # passes (hh + hl + lh) leave only the ~2^-16 ql*kl term off a
            # full fp32 matmul, and bf16 MMs stream with LDWEIGHTS hidden.
            qTh = main.tile([P, LQ], BF16, tag="qTh")
            qTl = main.tile([P, LQ], BF16, tag="qTl")
            kmTh = main.tile([P, NKB // 2, P], BF16, tag="kmTh")
            kmTl = main.tile([P, NKB // 2, P], BF16, tag="kmTl")
            nc.vector.tensor_copy(qTh, qT)
            nc.vector.tensor_sub(qTl, qT, qTh)
            kmT_f = kmT.rearrange("p a b -> p (a b)")
            kmTh_f = kmTh.rearrange("p a b -> p (a b)")
            nc.vector.tensor_copy(kmTh_f, kmT_f)
            nc.vector.tensor_sub(
                kmTl.rearrange("p a b -> p (a b)"), kmT_f, kmTh_f
            )

        out_sb = outp.tile([P, NQB, D], F32, tag="osb")

        # ---------------- main stream: S^T matmul -> exp -> PV ----------------
        for h in range(2):  # q halves of 1024
            pv = ps_pv.tile([P, 1024], F32, tag="pv")
            for j in range(NKB // 2):
                sA = ps_s.tile([P, 1024], F32, tag="s")
                sB = ps_s.tile([P, 1024], F32, tag="s")
                for c in range(2):  # 512-wide q slices
                    qs = slice(h * 1024 + c * 512, h * 1024 + (c + 1) * 512)
                    cs = slice(c * 512, (c + 1) * 512)
                    if three:
                        passes = [
                            (kmTh, qTh, True, False),
                            (kmTl, qTh, False, False),
                            (kmTh, qTl, False, True),
                        ]
                    else:
                        passes = [(kmT, qT, True, True)]
                    for kt, qt, st, sp in passes:
                        nc.tensor.matmul(
                            sA[:, cs], kt[0:64, j, :], qt[0:64, qs],
                            start=st, stop=sp, tile_position=(0, 0),
                        )
                        nc.tensor.matmul(
                            sB[:, cs], kt[64:128, j, :], qt[64:128, qs],
                            start=st, stop=sp, tile_position=(64, 0),
                        )
                wdt = F32R if PV_MODE == "f32r" else F32
                wA = wtp.tile([P, 1024], wdt, tag="wt")
                wB = wtp.tile([P, 1024], wdt, tag="wt")
                nc.scalar.activation(out=wA, in_=sA, func=EXP)
                nc.scalar.activation(out=wB, in_=sB, func=EXP)
                for c in range(2):
                    cs = slice(c * 512, (c + 1) * 512)
                    for kb, w in ((2 * j, wA), (2 * j + 1, wB)):
                        nc.tensor.matmul(
                            pv[0 : D + 1, cs], vme[:, kb, :], w[:, cs],
                            start=(kb == 0), stop=(kb == NKB - 1),
                        )

            # ---------------- drain: transpose back + normalize ----------------
            outT = outp.tile([D + 1, 1024], F32, tag="outT")
            nc.vector.tensor_copy(outT, pv[0 : D + 1, :])
            for qb in range(8):
                nat = ps_pv.tile([P, D + 1], F32, tag="pv")
                nc.tensor.transpose(
                    nat, outT[:, qb * P : (qb + 1) * P], ident[0 : D + 1, 0 : D + 1]
                )
                rc = smalls.tile([P, 1], F32, tag="rc")
                nc.vector.reciprocal(rc, nat[:, D : D + 1])
                nc.vector.tensor_scalar_mul(out_sb[:, h * 8 + qb, :], nat[:, 0:D], rc)

        nc.sync.dma_start(
            out=o_d[b].rearrange("(t p) d -> p t d", p=P), in_=out_sb
        )

    for p in reversed(ctx_pools):
        p.release()


_NC_CACHE = None


def _build_nc():
    global _NC_CACHE
    if _NC_CACHE is not None:
        return _NC_CACHE
    nc = bacc.Bacc(None, target_bir_lowering=False, debug=False)
    q_d = nc.dram_tensor("q", [PB, LQ, D], F32, kind="ExternalInput")
    k_d = nc.dram_tensor("k", [PB, LK, D], F32, kind="ExternalInput")
    v_d = nc.dram_tensor("v", [PB, LK, D], F32, kind="ExternalInput")
    m_d = nc.dram_tensor("m", [PB, LK], F32, kind="ExternalInput")
    o_d = nc.dram_tensor("out", [PB, LQ, D], F32, kind="ExternalOutput")
    with tile.TileContext(nc) as tc:
        _attention_core(tc, q_d, k_d, v_d, m_d, o_d)
    nc.compile()
    _NC_CACHE = nc
    return nc


def kernel(q, k, v, v_mask, _trace=False, _tmpdir=None):
    q = np.ascontiguousarray(q, dtype=np.float32)
    k = np.ascontiguousarray(k, dtype=np.float32)
    v = np.ascontiguousarray(v, dtype=np.float32)
    v_mask = np.ascontiguousarray(v_mask, dtype=np.float32)
    assert q.shape == (B, LQ, D), q.shape

    nc = _build_nc()
    in_maps = [
        {
            "q": q[i * PB : (i + 1) * PB],
            "k": k[i * PB : (i + 1) * PB],
            "v": v[i * PB : (i + 1) * PB],
            "m": v_mask[i * PB : (i + 1) * PB],
        }
        for i in range(NCORES)
    ]
    res = bass_utils.run_bass_kernel_spmd(
        nc, in_maps, core_ids=list(range(NCORES)), trace=_trace, tmpdir=_tmpdir
    )
    out = np.concatenate([r["out"] for r in res.results], axis=0)
    if _trace:
        kernel.last_results = res
    return out
